# revision 1
# baseline (speedup 1.0000x reference)
"""GQA kernel for 8 trn2 NeuronCores.

Sharding: tensor-parallel over heads. Core c owns KV head c and Q heads
4c..4c+3 (q-dim cols 256c:256c+256 of Wq, col 64c:64c+64 of Wk/Wv, rows
256c:256c+256 of Wo). Each core computes a partial output [B,S,E]
(its ctx slice @ its Wo row-slice); host sums the 8 partials.

Device algorithm (per core, per batch) — v2:
  A1. Q.T = Wq_c.T @ X.T, emitted as two head-PAIR tiles [128, S]
      (heads 2p at partitions 0:64, 2p+1 at 64:128), scaled by 1/8.
  A2. K.T/V.T projections, two passes of 2 t-chunks (2 PSUM accs);
      K.T duplicated into partitions 64:128 (SBUF->SBUF DMA) so score
      matmuls can row-tile both heads of a pair.
      V.T -> V natural via DMA transpose + ones column -> V_aug [S,65].
  B.  per (pair p, q-chunk jq of 512):
        row-tiled score pair: S.T[kv,q] for heads 2p/2p+1 concurrently
        (tile_position rows 0:64 / 64:128), PSUM pair tiles [128,1024]
        exp on ScalarE (PSUM->SBUF bf16), [128,1024] spans per head
        ctx.T[0:65] += V_aug.T @ P.T  (row 64 = softmax denominator)
      normalize: DVE recip + gpsimd partition-broadcast + DVE mul.
  C.  out_partial = ctx.T.T @ Wo_c  (natural layout, bf16 DMA out)

All matmuls bf16 inputs / fp32 PSUM accumulation. PSUM banks:
acc(2) + ctx pair(2) + scores(4) = 8.
"""

import numpy as np
import ml_dtypes

B = 2
S = 2048
E = 2048
HD = 64          # head dim
HPC = 4          # q heads per core
NP = 2           # head pairs per core
QD = HPC * HD    # 256 per-core q dims
NCORES = 8
EC = E // 128    # 16 contraction chunks
NJQ = S // 512   # 4 q-chunks of 512
NKV = S // 128   # 16 kv chunks of 128
KVG = 2          # kv chunks per exp group
BF16 = ml_dtypes.bfloat16

_cache = {}


def _build():
    from contextlib import ExitStack
    from concourse import bacc, tile
    import concourse.mybir as mybir

    bf16 = mybir.dt.bfloat16
    f32 = mybir.dt.float32
    EXP = mybir.ActivationFunctionType.Exp

    nc = bacc.Bacc(
        "TRN2", target_bir_lowering=False, debug=False, num_devices=NCORES)
    qT_d = nc.declare_dram_parameter("qT", [B, E, S], bf16, isOutput=False)
    kT_d = nc.declare_dram_parameter("kT", [B, E, S], bf16, isOutput=False)
    vT_d = nc.declare_dram_parameter("vT", [B, E, S], bf16, isOutput=False)
    wq_d = nc.declare_dram_parameter("wq", [E, QD], bf16, isOutput=False)
    wk_d = nc.declare_dram_parameter("wk", [E, HD], bf16, isOutput=False)
    wv_d = nc.declare_dram_parameter("wv", [E, HD], bf16, isOutput=False)
    wo_d = nc.declare_dram_parameter("wo", [QD, E], bf16, isOutput=False)
    out_d = nc.declare_dram_parameter("out", [B, S, E], bf16, isOutput=True)

    with ExitStack() as ctx:
        tc = ctx.enter_context(tile.TileContext(nc))
        # ---- pools ----
        wpool = ctx.enter_context(tc.tile_pool(name="w", bufs=1))
        qin = ctx.enter_context(tc.tile_pool(name="qin", bufs=16))
        kvin = ctx.enter_context(tc.tile_pool(name="kvin", bufs=3))
        qts = ctx.enter_context(tc.tile_pool(name="qts", bufs=2))
        vnp = ctx.enter_context(tc.tile_pool(name="vnp", bufs=16))
        ptp = ctx.enter_context(tc.tile_pool(name="ptp", bufs=4))
        ostp = ctx.enter_context(tc.tile_pool(name="ostp", bufs=4))
        smp = ctx.enter_context(tc.tile_pool(name="smp", bufs=4))
        psa = ctx.enter_context(tc.tile_pool(name="psa", bufs=2, space="PSUM"))
        psc = ctx.enter_context(tc.tile_pool(name="psc", bufs=2, space="PSUM"))

        # ---- weights (loaded once) ----
        wq_sb = wpool.tile([128, EC, QD], bf16)
        nc.sync.dma_start(wq_sb[:], wq_d.rearrange("(c p) m -> p c m", p=128))
        wk_sb = wpool.tile([128, EC, HD], bf16)
        nc.sync.dma_start(wk_sb[:], wk_d.rearrange("(c p) m -> p c m", p=128))
        wv_sb = wpool.tile([128, EC, HD], bf16)
        nc.sync.dma_start(wv_sb[:], wv_d.rearrange("(c p) m -> p c m", p=128))
        wo_sb = wpool.tile([128, 2, E], bf16)
        nc.sync.dma_start(wo_sb[:], wo_d.rearrange("(c p) e -> p c e", p=128))

        def phase_A(b):
            # ---------- A1: Q.T as pair tiles [128, S] ----------
            qtiles = []
            for e in range(EC):
                qt = qin.tile([128, S], bf16, tag="qin", name="qt")
                nc.sync.dma_start(qt[:], qT_d[b, e * 128:(e + 1) * 128, :])
                qtiles.append(qt)
            qp_sb = [qts.tile([128, S], bf16, tag=f"qp{p}", name=f"qp{p}")
                     for p in range(NP)]
            for m in range(NP):
                for t in range(NJQ):
                    acc = psa.tile([128, 512], f32, tag="acc", name="acc")
                    for e in range(EC):
                        nc.tensor.matmul(
                            acc[:], lhsT=wq_sb[:, e, m * 128:(m + 1) * 128],
                            rhs=qtiles[e][:, t * 512:(t + 1) * 512],
                            start=(e == 0), stop=(e == EC - 1))
                    nc.vector.tensor_scalar_mul(
                        qp_sb[m][:, t * 512:(t + 1) * 512], acc[:], 0.125)

            # ---------- A2: K.T, V.T (col-packed PSUM, 2 passes) ----------
            kt2_sb = qts.tile([128, S], bf16, tag="kt2")
            vt_sb = qts.tile([64, S], bf16, tag="vt")
            for half in range(2):
                kvaccs = [psa.tile([128, 512], f32, tag="acc", name="kvacc")
                          for _ in range(2)]
                for e in range(EC):
                    kt_in = kvin.tile([128, S // 2], bf16, tag="ktin")
                    nc.sync.dma_start(
                        kt_in[:],
                        kT_d[b, e * 128:(e + 1) * 128,
                             half * 1024:(half + 1) * 1024])
                    vt_in = kvin.tile([128, S // 2], bf16, tag="vtin")
                    nc.sync.dma_start(
                        vt_in[:],
                        vT_d[b, e * 128:(e + 1) * 128,
                             half * 1024:(half + 1) * 1024])
                    for ti in range(2):
                        nc.tensor.matmul(
                            kvaccs[ti][0:64, :], lhsT=wk_sb[:, e, :],
                            rhs=kt_in[:, ti * 512:(ti + 1) * 512],
                            start=(e == 0), stop=(e == EC - 1))
                        nc.tensor.matmul(
                            kvaccs[ti][64:128, :], lhsT=wv_sb[:, e, :],
                            rhs=vt_in[:, ti * 512:(ti + 1) * 512],
                            start=(e == 0), stop=(e == EC - 1),
                            tile_position=(0, 64))
                for ti in range(2):
                    t = half * 2 + ti
                    nc.vector.tensor_copy(
                        kt2_sb[0:64, t * 512:(t + 1) * 512], kvaccs[ti][0:64, :])
                    nc.vector.tensor_copy(
                        vt_sb[:, t * 512:(t + 1) * 512], kvaccs[ti][64:128, :])
            # duplicate K.T into partitions 64:128 (row-tiled score pairs)
            nc.sync.dma_start(kt2_sb[64:128, :], kt2_sb[0:64, :])

            # V natural + ones column -> V_aug [S, 65]
            vn_tiles = []
            for c in range(NKV):
                vn = vnp.tile([128, HD + 1], bf16, tag="vn", name="vn")
                nc.vector.memset(vn[:, HD:HD + 1], 1.0)
                nc.sync.dma_start_transpose(
                    out=vn[:, 0:HD], in_=vt_sb[0:64, c * 128:(c + 1) * 128])
                vn_tiles.append(vn)
            return qp_sb, kt2_sb, vn_tiles

        def phase_B(b, qp_sb, kt2_sb, vn_tiles):
            ctxT_sb = [qts.tile([128, S], bf16, tag=f"ctxT{i}", name=f"ctxT{i}")
                       for i in range(NP)]
            for p in range(NP):
                for jq in range(NJQ):
                    ctx_ps = psa.tile([128, 1024], f32, tag="ctx", bufs=1,
                                      name="ctx_ps")
                    for g in range(NKV // KVG):
                        sc_e = psc.tile([128, KVG * 512], f32, tag="sc",
                                        name="sc_e")
                        sc_o = psc.tile([128, KVG * 512], f32, tag="sc",
                                        name="sc_o")
                        for ki in range(KVG):
                            kv = g * KVG + ki
                            nc.tensor.matmul(
                                sc_e[:, ki * 512:(ki + 1) * 512],
                                lhsT=kt2_sb[0:64, kv * 128:(kv + 1) * 128],
                                rhs=qp_sb[p][0:64, jq * 512:(jq + 1) * 512],
                                start=True, stop=True)
                            nc.tensor.matmul(
                                sc_o[:, ki * 512:(ki + 1) * 512],
                                lhsT=kt2_sb[64:128, kv * 128:(kv + 1) * 128],
                                rhs=qp_sb[p][64:128, jq * 512:(jq + 1) * 512],
                                start=True, stop=True)
                        pt_e = ptp.tile([128, KVG * 512], bf16, tag="pt",
                                        name="pt_e")
                        nc.scalar.activation(pt_e[:], sc_e[:], EXP)
                        pt_o = ptp.tile([128, KVG * 512], bf16, tag="pt",
                                        name="pt_o")
                        nc.scalar.activation(pt_o[:], sc_o[:], EXP)
                        for ki in range(KVG):
                            kv = g * KVG + ki
                            nc.tensor.matmul(
                                ctx_ps[0:HD + 1, 0:512],
                                lhsT=vn_tiles[kv][:, 0:HD + 1],
                                rhs=pt_e[:, ki * 512:(ki + 1) * 512],
                                start=(kv == 0), stop=(kv == NKV - 1))
                            nc.tensor.matmul(
                                ctx_ps[0:HD + 1, 512:1024],
                                lhsT=vn_tiles[kv][:, 0:HD + 1],
                                rhs=pt_o[:, ki * 512:(ki + 1) * 512],
                                start=(kv == 0), stop=(kv == NKV - 1))
                    # normalize both heads by their softmax denominators
                    for hp in range(2):
                        recip = smp.tile([1, 512], f32, tag="recip",
                                         name="recip")
                        nc.vector.reciprocal(
                            recip[:], ctx_ps[HD:HD + 1, hp * 512:(hp + 1) * 512])
                        rb = smp.tile([64, 512], f32, tag="rb", name="rb")
                        nc.gpsimd.partition_broadcast(rb[:], recip[:])
                        nc.vector.tensor_mul(
                            ctxT_sb[p][hp * 64:(hp + 1) * 64,
                                       jq * 512:(jq + 1) * 512],
                            ctx_ps[0:64, hp * 512:(hp + 1) * 512], rb[:])
            return ctxT_sb

        def phase_C(b, ctxT_sb):
            for t in range(S // 128):
                for e in range(E // 512):
                    ops = psa.tile([128, 512], f32, tag="acc", name="ops")
                    for kc in range(2):
                        nc.tensor.matmul(
                            ops[:], lhsT=ctxT_sb[kc][:, t * 128:(t + 1) * 128],
                            rhs=wo_sb[:, kc, e * 512:(e + 1) * 512],
                            start=(kc == 0), stop=(kc == 1))
                    ost = ostp.tile([128, 512], bf16, tag="ost", name="ost")
                    nc.vector.tensor_copy(ost[:], ops[:])
                    nc.sync.dma_start(
                        out_d[b, t * 128:(t + 1) * 128, e * 512:(e + 1) * 512],
                        ost[:])

        # software-pipelined emission order: A0 B0 A1 C0 B1 C1
        st0 = phase_A(0)
        ctxT0 = phase_B(0, *st0)
        st1 = phase_A(1)
        phase_C(0, ctxT0)
        ctxT1 = phase_B(1, *st1)
        phase_C(1, ctxT1)
    nc.compile()
    return nc


def _get_nc():
    if "nc" not in _cache:
        _cache["nc"] = _build()
    return _cache["nc"]


def kernel(query, key, value, Wq, Wk, Wv, Wo, _trace=False):
    from concourse.bass_utils import run_bass_kernel_spmd

    def t_bf16(x):
        return np.ascontiguousarray(
            np.asarray(x, np.float32).astype(BF16).transpose(0, 2, 1))

    qT = t_bf16(query)
    kT = t_bf16(key)
    vT = t_bf16(value)
    Wq = np.asarray(Wq, np.float32).astype(BF16)
    Wk = np.asarray(Wk, np.float32).astype(BF16)
    Wv = np.asarray(Wv, np.float32).astype(BF16)
    Wo = np.asarray(Wo, np.float32).astype(BF16)

    in_maps = []
    for c in range(NCORES):
        in_maps.append({
            "qT": qT, "kT": kT, "vT": vT,
            "wq": np.ascontiguousarray(Wq[:, c * QD:(c + 1) * QD]),
            "wk": np.ascontiguousarray(Wk[:, c * HD:(c + 1) * HD]),
            "wv": np.ascontiguousarray(Wv[:, c * HD:(c + 1) * HD]),
            "wo": np.ascontiguousarray(Wo[c * QD:(c + 1) * QD, :]),
        })

    nc = _get_nc()
    res = run_bass_kernel_spmd(nc, in_maps, list(range(NCORES)), trace=_trace)
    out = res.results[0]["out"].astype(np.float32)
    for c in range(1, NCORES):
        out += res.results[c]["out"].astype(np.float32)
    if _trace:
        _cache["last_exec_time_ns"] = res.exec_time_ns
        _cache["last_results"] = res
    return out

